# revision 26
# baseline (speedup 1.0000x reference)
"""AxialAttention (MSA row attention) on 8 Trainium2 NeuronCores.

Sharding: data parallel over MSA rows r=128 (16 rows/core); the edge-bias
precompute is sharded over the edge i dim (32 i-rows/core) in kernel 1,
post-processed + gathered on host, replicated into kernel 2.

v2 design (vs baseline): all matmuls in bf16; dense 4-heads-per-128 slot
layout (2 blocks) via explicit tile_position; whole-core phases
(LN+transpose -> weight-stationary projections -> per-row attention) so the
scalar engine only swaps activation tables twice (sqrt -> sigmoid -> exp);
denominators land densely on the same partitions as attn@v via ones-matmuls
at 32-row col positions, so softmax normalize/gate runs as full-width
[128,256] vector ops; bias kernel defers LayerNorm algebra to the host
(device emits raw edge@W products + mean + sumsq via matmuls only).
"""

import sys
import numpy as np
import ml_dtypes

sys.path.insert(0, "/opt/trn_rl_repo")

import concourse.bacc as bacc
import concourse.tile as tile
import concourse.bass as bass
from concourse import mybir
from concourse import bass_utils

F32 = mybir.dt.float32
F32R = mybir.dt.float32r
BF16 = mybir.dt.bfloat16
U8 = mybir.dt.uint8
AF = mybir.ActivationFunctionType
MUL = mybir.AluOpType.mult

NC = 8          # cores
B, R, W, DN = 1, 128, 256, 256
DE, H, DH = 128, 8, 32
RPC = R // NC   # rows per core = 16
IPC = W // NC   # edge i-rows per core = 32
NEG = -1.0e38
EPS = 1e-5

NB = 2                      # head blocks, 4 heads each (dense)
SLOTS = NB * 128            # 256
P = 128
TOK = RPC * W               # 4096 tokens per core
NT = TOK // P               # 32 token tiles


def _head_slot(h):
    return (h // 4) * 128 + 32 * (h % 4)


def _expand_cols(Wm):
    D = Wm.shape[0]
    out = np.zeros((D, SLOTS), Wm.dtype)
    for h in range(H):
        out[:, _head_slot(h):_head_slot(h) + DH] = Wm[:, h * DH:(h + 1) * DH]
    return out


def _expand_rows(Wm):
    D = Wm.shape[1]
    out = np.zeros((SLOTS, D), Wm.dtype)
    for h in range(H):
        out[_head_slot(h):_head_slot(h) + DH, :] = Wm[h * DH:(h + 1) * DH, :]
    return out


# ---------------------------------------------------------------- kernel 1
def _build_bias_nc():
    """Per core: pre-transposed edges slice eT [DE, IPC*W] (bf16, host
    transposes for free) -> raw [10, IPC*W]: rows 0:8 = sum_e e*we[e,h];
    row 8 = mean_e e; row 9 = sum_e e^2. LN algebra on host."""
    nc = bacc.Bacc("TRN2", target_bir_lowering=False, debug=False,
                   num_devices=NC)
    TOKE = IPC * W  # 8192
    e_d = nc.dram_tensor("e", [DE, TOKE], BF16, kind="ExternalInput").ap()
    wa_d = nc.dram_tensor("wa", [DE, 9], BF16, kind="ExternalInput").ap()
    o_d = nc.dram_tensor("o", [10, TOKE], F32, kind="ExternalOutput").ap()

    with tile.TileContext(nc) as tc:
        with tc.tile_pool(name="cst", bufs=1) as cst, \
             tc.tile_pool(name="work", bufs=4) as work, \
             tc.tile_pool(name="psr", bufs=4, space="PSUM") as psr:
            wa_sb = cst.tile([DE, 9], BF16)
            nc.sync.dma_start(out=wa_sb, in_=wa_d)
            onesc = cst.tile([P, 1], BF16)
            nc.vector.memset(onesc, 1.0)
            eT_all = cst.tile([P, TOKE], BF16)
            qeng = [nc.sync, nc.scalar, nc.gpsimd, nc.sync]
            QN = TOKE // 4
            for h in range(4):
                qeng[h].dma_start(out=eT_all[:, h * QN:(h + 1) * QN],
                                  in_=e_d[:, h * QN:(h + 1) * QN])

            for g in range(TOKE // 512):
                eT = eT_all[:, g * 512:(g + 1) * 512]
                sq = work.tile([P, 512], BF16, tag="sq")
                nc.gpsimd.tensor_tensor(out=sq, in0=eT, in1=eT, op=MUL)
                raw_ps = psr.tile([P, 512], F32, tag="raw")
                nc.tensor.matmul(raw_ps[0:9, :], wa_sb[:], eT,
                                 start=True, stop=True)
                nc.tensor.matmul(raw_ps[32:33, :], onesc[:], sq[:],
                                 start=True, stop=True,
                                 tile_position=(0, 32))
                rawsb = work.tile([P, 512], F32, tag="rawsb")
                if g % 2 == 0:
                    nc.scalar.copy(rawsb[0:9, :], raw_ps[0:9, :])
                    nc.scalar.copy(rawsb[32:33, :], raw_ps[32:33, :])
                else:
                    nc.vector.tensor_copy(out=rawsb[0:9, :],
                                          in_=raw_ps[0:9, :])
                    nc.vector.tensor_copy(out=rawsb[32:33, :],
                                          in_=raw_ps[32:33, :])
                eng2 = nc.scalar if g % 2 == 0 else nc.sync
                eng2.dma_start(out=o_d[0:9, g * 512:(g + 1) * 512],
                               in_=rawsb[0:9, :])
                eng2.dma_start(out=o_d[9:10, g * 512:(g + 1) * 512],
                               in_=rawsb[32:33, :])
    nc.compile()
    return nc


# ---------------------------------------------------------------- kernel 2
def _build_attn_nc():
    nc = bacc.Bacc("TRN2", target_bir_lowering=False, debug=False,
                   num_devices=NC)

    x_d = nc.dram_tensor("x", [TOK, DN], BF16, kind="ExternalInput").ap()
    wq_d = nc.dram_tensor("wq", [DN, SLOTS], BF16, kind="ExternalInput").ap()
    wk_d = nc.dram_tensor("wk", [DN, SLOTS], BF16, kind="ExternalInput").ap()
    wv_d = nc.dram_tensor("wv", [DN, SLOTS], BF16, kind="ExternalInput").ap()
    wg_d = nc.dram_tensor("wg", [DN, SLOTS], BF16, kind="ExternalInput").ap()
    wo_d = nc.dram_tensor("wo", [SLOTS, DN], BF16, kind="ExternalInput").ap()
    bg_d = nc.dram_tensor("bg", [P, NB], F32, kind="ExternalInput").ap()
    bo_d = nc.dram_tensor("bo", [1, DN], BF16, kind="ExternalInput").ap()
    bt_d = nc.dram_tensor("bt", [P, H, 2, W], BF16,
                          kind="ExternalInput").ap()
    id_d = nc.dram_tensor("idm", [P, P], BF16, kind="ExternalInput").ap()
    on_d = nc.dram_tensor("onesb", [P, P], BF16, kind="ExternalInput").ap()
    ngj_d = nc.dram_tensor("ngj", [P, RPC * 2], F32,
                           kind="ExternalInput").ap()
    invm_d = nc.dram_tensor("invm", [RPC, W], U8, kind="ExternalInput").ap()
    o_d = nc.dram_tensor("o", [TOK, DN], BF16, kind="ExternalOutput").ap()
    if DBG:
        dbg_d = nc.dram_tensor("dbg", [P, 3072], F32,
                               kind="ExternalOutput").ap()

    with tile.TileContext(nc, trace_sim=SIM_TRACE) as tc:
        from contextlib import ExitStack
        with ExitStack() as ctx:
            cst = ctx.enter_context(tc.tile_pool(name="cst", bufs=1))

            # ---------------- persistent tiles
            ident = cst.tile([P, P], BF16)
            nc.sync.dma_start(out=ident, in_=id_d)
            ones_sq = cst.tile([P, P], BF16)
            nc.sync.dma_start(out=ones_sq, in_=on_d)
            ones_blk = ones_sq[:, 0:32]
            one1 = ones_sq[0:1, :]
            onecol = ones_sq[:, 0:1]
            eps_sb = cst.tile([P, 1], F32)
            nc.vector.memset(eps_sb, EPS)

            def load_w(d, shape, nm, dt=BF16):
                t = cst.tile(shape, dt, tag=nm, name=nm)
                nc.sync.dma_start(out=t, in_=d)
                return t

            wq = [load_w(wq_d[kt * P:(kt + 1) * P, :], [P, SLOTS], f"wq{kt}")
                  for kt in range(2)]
            wk = [load_w(wk_d[kt * P:(kt + 1) * P, :], [P, SLOTS], f"wk{kt}")
                  for kt in range(2)]
            wv = [load_w(wv_d[kt * P:(kt + 1) * P, :], [P, SLOTS], f"wv{kt}")
                  for kt in range(2)]
            wg = [load_w(wg_d[kt * P:(kt + 1) * P, :], [P, SLOTS], f"wg{kt}")
                  for kt in range(2)]
            wo = [load_w(wo_d[b * P:(b + 1) * P, :], [P, DN], f"wo{b}")
                  for b in range(NB)]
            bg = load_w(bg_d, [P, NB], "bgt", F32)
            bo = load_w(bo_d, [1, DN], "bot")
            bt_sb = load_w(bt_d, [P, H, 2, W], "btt")
            ngj = load_w(ngj_d, [P, RPC * 2], "ngjt", F32)
            invm_all = cst.tile([P, RPC * W], U8, tag="invm", name="invm")
            nc.sync.dma_start(
                out=invm_all,
                in_=bass.AP(tensor=invm_d.tensor, offset=0,
                            ap=[[0, P], [1, RPC * W]]))

            xnT = [cst.tile([P, TOK], BF16, tag=f"xnT{kt}", name=f"xnT{kt}")
                   for kt in range(2)]
            q_sb = [cst.tile([P, TOK], BF16, tag=f"q{b}", name=f"q{b}")
                    for b in range(NB)]
            k_sb = [cst.tile([P, TOK], BF16, tag=f"k{b}", name=f"k{b}")
                    for b in range(NB)]
            sig_sb = [cst.tile([P, TOK], BF16, tag=f"sg{b}", name=f"sg{b}")
                      for b in range(NB)]
            v_sb = [cst.tile([P, SLOTS], BF16, tag=f"v{tt}", name=f"v{tt}")
                    for tt in range(NT)]
            vbarW_all = cst.tile([P, 2 * RPC], F32, tag="vbw_all",
                                 name="vbw_all")

            # ---------------- phase A: LN + transpose (sqrt act table)
            with tc.tile_pool(name="lnw", bufs=4) as lnw, \
                 tc.tile_pool(name="xbp", bufs=2) as xbp, \
                 tc.tile_pool(name="tpp", bufs=2, space="PSUM") as tpp, \
                 tc.tile_pool(name="pjp", bufs=3, space="PSUM") as pjp:
                for g in range(NT // 4):
                    xb = xbp.tile([P, 4 * DN], BF16, tag="xb", name=f"xb{g}")
                    (nc.sync if g % 2 == 0 else nc.gpsimd).dma_start(
                        out=xb,
                        in_=bass.AP(tensor=x_d.tensor, offset=g * 4 * P * DN,
                                    ap=[[DN, P], [P * DN, 4], [1, DN]]))
                    tp_ps = tpp.tile([P, 1024], BF16, tag="tp",
                                     name=f"tp{g}")
                    for c in range(4):
                        xt = xb[:, c * DN:(c + 1) * DN]
                        stats = lnw.tile([P, 6], F32, tag="st")
                        nc.vector.bn_stats(out=stats, in_=xt)
                        mv = lnw.tile([P, 2], F32, tag="mv")
                        nc.vector.bn_aggr(out=mv, in_=stats)
                        sd = lnw.tile([P, 1], F32, tag="sd")
                        nc.scalar.activation(sd, mv[:, 1:2], AF.Sqrt,
                                             bias=eps_sb[:])
                        rstd = lnw.tile([P, 1], F32, tag="rs")
                        nc.vector.reciprocal(rstd, sd)
                        nmr = lnw.tile([P, 1], F32, tag="nm")
                        nc.vector.scalar_tensor_tensor(
                            out=nmr, in0=mv[:, 0:1], scalar=-1.0, in1=rstd,
                            op0=MUL, op1=MUL)
                        xn = lnw.tile([P, DN], BF16, tag="xn")
                        nc.scalar.activation(xn, xt, AF.Identity,
                                             bias=nmr[:], scale=rstd[:])
                        for kt in range(2):
                            nc.tensor.transpose(
                                tp_ps[:, kt * 512 + c * P:
                                      kt * 512 + (c + 1) * P],
                                xn[:, kt * P:(kt + 1) * P], ident[:])
                    for kt in range(2):
                        nc.vector.tensor_copy(
                            out=xnT[kt][:, g * 512:(g + 1) * 512],
                            in_=tp_ps[:, kt * 512:(kt + 1) * 512])

                # ---------------- phase B: projections
                # g first (sigmoid table), then q/k/v (copy, table-free)
                def proj_qkg(ws, b, ch, pp):
                    pass
                def proj_qkg(ws, b, ch, pp):
                    for kt in range(2):
                        nc.tensor.matmul(
                            pp[:], ws[kt][:, b * P:(b + 1) * P],
                            xnT[kt][:, ch * 512:(ch + 1) * 512],
                            start=(kt == 0), stop=(kt == 1))

                for b in range(NB):
                    for ch in range(TOK // 512):
                        pp = pjp.tile([P, 512], F32, tag="pj")
                        proj_qkg(wg, b, ch, pp)
                        nc.scalar.activation(
                            sig_sb[b][:, ch * 512:(ch + 1) * 512], pp,
                            AF.Sigmoid, bias=bg[:, b:b + 1])
                for b in range(NB):
                    for ch in range(TOK // 512):
                        pp = pjp.tile([P, 512], F32, tag="pj")
                        proj_qkg(wq, b, ch, pp)
                        nc.scalar.copy(
                            q_sb[b][:, ch * 512:(ch + 1) * 512], pp)
                for b in range(NB):
                    for ch in range(TOK // 512):
                        pp = pjp.tile([P, 512], F32, tag="pj")
                        proj_qkg(wk, b, ch, pp)
                        nc.vector.tensor_copy(
                            out=k_sb[b][:, ch * 512:(ch + 1) * 512], in_=pp)
                for tt in range(NT):
                    pp = pjp.tile([P, SLOTS], F32, tag="vps", bufs=2)
                    for kt in range(2):
                        nc.tensor.matmul(
                            pp[:], xnT[kt][:, tt * P:(tt + 1) * P],
                            wv[kt][:], start=(kt == 0), stop=(kt == 1))
                    if tt % 2 == 0:
                        nc.scalar.copy(v_sb[tt][:], pp)
                    else:
                        nc.vector.tensor_copy(out=v_sb[tt][:], in_=pp)

            # ---------------- phase C: per-row attention (exp table)
            with tc.tile_pool(name="etp", bufs=4) as etp, \
                 tc.tile_pool(name="rowp", bufs=3) as rowp, \
                 tc.tile_pool(name="ogp", bufs=4) as ogp, \
                 tc.tile_pool(name="dtp", bufs=2, space="PSUM") as dtp, \
                 tc.tile_pool(name="adp", bufs=1, space="PSUM") as adp, \
                 tc.tile_pool(name="opp", bufs=2, space="PSUM") as opp,:
                for r in range(DO_C_ROWS):
                    t0 = r * W

                    ogrs = []
                    for b in range(NB if C_LEVEL >= 2 else 0):
                        avdn = adp.tile([P, 512], F32, tag="ad")
                        ets = []
                        for jt in range(2):
                            dt_ps = dtp.tile([P, 1024], F32, tag="dt")
                            # NB: the full-row bias matmul must sit between
                            # consecutive 32-row QK matmuls: back-to-back
                            # matmuls at different row tile positions hang
                            # the PE (probe4 qk2 vs pair).
                            for u in range(4):
                                h = 4 * b + u
                                ho = 32 * u
                                nc.tensor.matmul(
                                    dt_ps[:, u * W:(u + 1) * W],
                                    ident[:], bt_sb[:, h, jt, :],
                                    start=True, stop=False)
                                nc.tensor.matmul(
                                    dt_ps[:, u * W:(u + 1) * W],
                                    k_sb[b][ho:ho + DH,
                                            t0 + jt * P:t0 + (jt + 1) * P],
                                    q_sb[b][ho:ho + DH, t0:t0 + W],
                                    start=False, stop=True,
                                    tile_position=(ho, 0))
                            et = etp.tile([P, 1024], BF16, tag="et")
                            nc.scalar.activation(
                                et, dt_ps, AF.Exp,
                                bias=ngj[:, r * 2 + jt:r * 2 + jt + 1])
                            ets.append(et)
                        # each PSUM accumulation group must run start->stop
                        # contiguously; interleaving groups at different
                        # tile positions corrupts the accumulation
                        # (probe5 full vs seq).
                        for u in range(4 if C_LEVEL >= 3 else 0):
                            ho = 32 * u
                            for jt in range(2):
                                nc.tensor.matmul(
                                    avdn[ho:ho + 32, 0:W],
                                    v_sb[2 * r + jt][:, b * P + ho:
                                                     b * P + ho + DH],
                                    ets[jt][:, u * W:(u + 1) * W],
                                    start=(jt == 0), stop=(jt == 1),
                                    tile_position=(0, ho))
                            for jt in range(2):
                                nc.tensor.matmul(
                                    avdn[ho:ho + 32, W:2 * W],
                                    ones_blk,
                                    ets[jt][:, u * W:(u + 1) * W],
                                    start=(jt == 0), stop=(jt == 1),
                                    tile_position=(0, ho))
                        if C_LEVEL < 4:
                            continue
                        if DBG and r == 0 and b == 0:
                            dv = rowp.tile([P, 3072], F32, tag="dbgv")
                            nc.vector.tensor_copy(out=dv[:, 0:256],
                                                  in_=q_sb[0][:, 0:256])
                            nc.vector.tensor_copy(out=dv[:, 256:512],
                                                  in_=k_sb[0][:, 0:256])
                            nc.vector.tensor_copy(out=dv[:, 512:768],
                                                  in_=sig_sb[0][:, 0:256])
                            nc.vector.tensor_copy(out=dv[:, 768:1024],
                                                  in_=v_sb[0][:])
                            nc.vector.tensor_copy(out=dv[:, 1024:2048],
                                                  in_=ets[0][:])
                            nc.vector.tensor_copy(out=dv[:, 2048:2560],
                                                  in_=avdn[:])
                            nc.vector.tensor_copy(out=dv[:, 2560:2816],
                                                  in_=xnT[0][:, 0:256])
                            nc.sync.dma_start(out=dbg_d, in_=dv)
                        rbig = rowp.tile([P, W], F32, tag="rbig")
                        nc.vector.reciprocal_approx_fast(rbig, avdn[:, W:])
                        t1 = rowp.tile([P, W], F32, tag="t1")
                        nc.vector.tensor_tensor(out=t1, in0=avdn[:, 0:W],
                                                in1=rbig, op=MUL)
                        ogr = ogp.tile([P, W], BF16, tag=f"og{b}")
                        nc.vector.tensor_tensor(
                            out=ogr, in0=t1,
                            in1=sig_sb[b][:, t0:t0 + W], op=MUL)
                        vbs = rowp.tile([P, W], BF16, tag="vbs")
                        nc.vector.tensor_scalar(
                            out=vbs, in0=sig_sb[b][:, t0:t0 + W],
                            scalar1=vbarW_all[:, 2 * r + b:2 * r + b + 1], scalar2=None,
                            op0=MUL)
                        nc.vector.copy_predicated(
                            out=ogr, mask=invm_all[:, t0:t0 + W], data=vbs)
                        ogrs.append(ogr)

                    for it in range(2 if C_LEVEL >= 5 else 0):
                        op_ps = opp.tile([P, DN], F32, tag="op")
                        nc.tensor.matmul(op_ps[:], one1, bo[:],
                                         start=True, stop=False)
                        for b in range(NB):
                            nc.tensor.matmul(
                                op_ps[:], ogrs[b][:, it * P:(it + 1) * P],
                                wo[b][:], start=False, stop=(b == NB - 1))
                        ot = rowp.tile([P, DN], BF16, tag="ot")
                        if it % 2 == 0:
                            nc.scalar.copy(ot, op_ps)
                        else:
                            nc.vector.tensor_copy(out=ot, in_=op_ps)
                        (nc.sync if (2 * r + it) % 2 == 0
                         else nc.scalar).dma_start(
                            out=o_d[t0 + it * P:t0 + (it + 1) * P, :],
                            in_=ot)
    nc.compile()
    return nc


_NC_CACHE = {}
TRACE = False
SIM_TRACE = False
DO_B = True
DO_C_ROWS = RPC
DBG = False
C_LEVEL = 5  # 1=vbar 2=+dots/exp 3=+avsum 4=+og 5=+outproj


def _get_nc(name):
    if name not in _NC_CACHE:
        _NC_CACHE[name] = (_build_bias_nc if name == "bias"
                           else _build_attn_nc)()
    return _NC_CACHE[name]


def _prep(x, edges, mask, edge_mask, ln_g, ln_b, lne_g, lne_b,
          W_edge, Wq, Wkv, Wg, bg, Wo, bo):
    f32 = np.float32
    bf16 = ml_dtypes.bfloat16
    x = np.asarray(x, f32)
    edges = np.asarray(edges, f32)
    mask_b = np.asarray(mask).astype(bool)
    edge_mask_b = np.asarray(edge_mask).astype(bool)
    ln_g = np.asarray(ln_g, f32); ln_b = np.asarray(ln_b, f32)
    lne_g = np.asarray(lne_g, f32); lne_b = np.asarray(lne_b, f32)
    W_edge = np.asarray(W_edge, f32)
    Wq = np.asarray(Wq, f32); Wkv = np.asarray(Wkv, f32)
    Wg = np.asarray(Wg, f32); bg = np.asarray(bg, f32)
    Wo = np.asarray(Wo, f32); bo = np.asarray(bo, f32)

    # ---------------- kernel 1: raw edge products
    nc1 = _get_nc("bias")
    we = (lne_g[:, None] * W_edge).astype(f32)
    we_bf = we.astype(bf16)
    wa = np.zeros((DE, 9), f32)
    wa[:, 0:8] = we_bf.astype(f32)
    wa[:, 8] = 1.0 / DE
    e_flat = edges.reshape(W, W, DE)
    in_maps1 = []
    for c in range(NC):
        in_maps1.append({
            "e": np.ascontiguousarray(
                e_flat[c * IPC:(c + 1) * IPC].reshape(IPC * W, DE).T
            ).astype(bf16),
            "wa": wa.astype(bf16),
        })
    res1 = bass_utils.run_bass_kernel_spmd(nc1, in_maps1,
                                           core_ids=list(range(NC)),
                                           trace=TRACE)
    if TRACE:
        print("bias kernel exec_time_ns:", res1.exec_time_ns)
    o1 = np.concatenate([res1.results[c]["o"] for c in range(NC)],
                        axis=1)  # [10, W*W]
    raw = o1[0:8]                        # [8, i*j]
    mu = o1[8]                           # [i*j]
    var = o1[9] / DE - mu * mu
    rstd = 1.0 / np.sqrt(var + EPS)
    swe = we_bf.astype(f32).sum(axis=0)  # [H]
    bias = rstd[None, :] * (raw - mu[None, :] * swe[:, None])
    bias = bias.reshape(H, W, W) + (lne_b @ W_edge)[:, None, None]
    bias = np.where(edge_mask_b[0][None], bias, NEG).astype(f32)
    biasT = np.ascontiguousarray(bias.transpose(0, 2, 1))  # [H, j, i]
    bt = np.ascontiguousarray(
        biasT.reshape(H, 2, P, W).transpose(2, 0, 1, 3))   # [128, H, 2, W]

    # ---------------- kernel 2: attention
    nc2 = _get_nc("attn")
    scale = DH ** -0.5
    Wk_, Wv_ = Wkv[:, :H * DH], Wkv[:, H * DH:]
    gq = _expand_cols((ln_g[:, None] * Wq * scale).astype(f32))
    gk = _expand_cols((ln_g[:, None] * Wk_).astype(f32))
    gv = _expand_cols((ln_g[:, None] * Wv_).astype(f32))
    gg = _expand_cols((ln_g[:, None] * Wg).astype(f32))
    assert np.allclose(ln_b, 0.0), "ln_b folding not implemented"
    bgx = np.zeros((P, NB), f32)
    for h in range(H):
        bgx[32 * (h % 4):32 * (h % 4) + DH, h // 4] = \
            bg[h * DH:(h + 1) * DH]
    woe = _expand_rows(Wo.astype(f32))

    maskf = mask_b[0].astype(f32)  # [R, W]
    x_flat = x.reshape(R, W, DN)
    in_maps2 = []
    for c in range(NC):
        mrows = maskf[c * RPC:(c + 1) * RPC]  # [RPC, W]
        ngj = (mrows.reshape(RPC, 2, P) - 1.0) * 1e38  # [r, jt, p]
        ngj = np.ascontiguousarray(
            ngj.transpose(2, 0, 1).reshape(P, RPC * 2))
        in_maps2.append({
            "x": np.ascontiguousarray(
                x_flat[c * RPC:(c + 1) * RPC].reshape(TOK, DN)
            ).astype(bf16),
            "wq": gq.astype(bf16), "wk": gk.astype(bf16),
            "wv": gv.astype(bf16), "wg": gg.astype(bf16),
            "wo": woe.astype(bf16),
            "bg": bgx, "bo": bo.reshape(1, DN).astype(bf16),
            "bt": bt.astype(bf16), "ngj": ngj.astype(f32),
            "idm": np.eye(P, dtype=f32).astype(bf16),
            "onesb": np.ones((P, P), bf16),
            "invm": (1.0 - mrows).astype(np.uint8),
        })
    return nc2, in_maps2


def kernel(**inputs):
    nc2, in_maps2 = _prep(**inputs)
    res2 = bass_utils.run_bass_kernel_spmd(nc2, in_maps2,
                                           core_ids=list(range(NC)),
                                           trace=TRACE)
    if TRACE:
        print("attn kernel exec_time_ns:", res2.exec_time_ns)
    out = np.concatenate(
        [res2.results[c]["o"].astype(np.float32).reshape(RPC, W, DN)
         for c in range(NC)],
        axis=0)
    return out.reshape(B, R, W, DN).astype(np.float32)
